# revision 42
# baseline (speedup 1.0000x reference)
"""Trainium2 Bass kernel for nn_EnsembleDynamicModel.

Ensemble MLP: E=7 members, x=[state(32)|action(8)] -> 256 -> 256 -> 256 -> 128
-> {mu(32), log_sigma(32)} with swish hidden activations, soft-clamped
log_sigma -> sigma=exp(.), and mu += state residual.

Strategy: data-parallel over the batch axis. Each of the 8 NeuronCores gets
B/8 = 4096 batch rows; ensemble weights are replicated. On-chip layout keeps
activations feature-major ([feature, batch]) so the contraction dim of every
GEMM sits on SBUF partitions.

The K=256 layers (L1/L2/L3) run as fp8e4m3 DoubleRow matmuls: weights and
hidden activations are stored fp8, with the two 128-row K-halves side by
side in the free dim ([128, 2, N] access patterns). DoubleRow contracts
K=256 in a single pass at 1 column/cycle - 2x the bf16 rate. L0 (K=40) and
the heads (K=128) gain nothing from DoubleRow and stay bf16/fp8-1x.
Accuracy: fp8 weights+activations on the hidden chain measures ~2e-3
scale-rel error vs the fp32 reference (gate is 2e-2).

Engines:
  PE   : GEMM chain, ~96us/core (vs ~155us bf16). No accumulation chains -
         every matmul is start=stop=True.
  ACT  : swish fused with the bias add (Silu(psum + b)) writing fp8 directly;
         the sigma head's sigmoid runs as Tanh (same ACT table set as Silu).
         ACT is the bottleneck engine (~190us busy): every cycle counts.
  DVE  : one fused affine_then_add drains each head psum (mu = psum + bmu +
         state on rows 0-31, sigma-preact + (bsig-max) on rows 32-63), plus
         the final sigma scale/offset.

The reference's soft_clamp+exp collapses exactly:
    sigma = exp(min) + exp(max) * sigmoid(y - max).

Ensembles are software-pipelined: L0(e+1) is emitted between L3(e) and
head(e) so the PE always has independent matmul work while head psums drain.
"""

import os
import sys
import numpy as np
from contextlib import ExitStack

# concourse ships with the container image (also on PYTHONPATH via axon_site).
for _p in ("/opt/trn_rl_repo", "/root/.axon_site/_ro/trn_rl_repo"):
    if os.path.isdir(_p) and _p not in sys.path:
        sys.path.append(_p)

import ml_dtypes  # noqa: E402
import concourse.bass as bass  # noqa: E402
import concourse.tile as tile  # noqa: E402
import concourse.mybir as mybir  # noqa: E402
from concourse import bacc  # noqa: E402
from concourse.bass_utils import run_bass_kernel_spmd  # noqa: E402
# NOTE: walrus's --enable-ldw-opt=true (LDWEIGHTS dedup) fails to compile
# fp8 weight loads; redundant reloads are hidden instead by streaming 512
# out-cols per DoubleRow matmul (4D moving APs).

F32 = mybir.dt.float32
BF16 = mybir.dt.bfloat16
F8 = mybir.dt.float8e4
AF = mybir.ActivationFunctionType
DR = mybir.MatmulPerfMode.DoubleRow

NP_BF16 = ml_dtypes.bfloat16
NP_F8 = ml_dtypes.float8_e4m3

E = 7
B = 32768
S = 32
A = 8
DIN = S + A            # 40
NCORES = 8
BL = B // NCORES       # 4096 batch rows per core
CH = 2048              # batch chunk per psum tile (4 PSUM banks fp32)
NSUB = 512             # bf16/1x matmul free dim (1 PSUM bank fp32)
NDR = 256              # DoubleRow out free dim (rhs moving = 2*NDR = 512)
NCHUNK = BL // CH      # 2
NCONST = 8             # const columns per ensemble member


def _build_kernel(ctx, tc, io, act=AF.Silu):
    nc = tc.nc
    cpool = ctx.enter_context(tc.tile_pool(name="cpool", bufs=1))
    hpool = ctx.enter_context(tc.tile_pool(name="hpool", bufs=1))
    wpool = ctx.enter_context(tc.tile_pool(name="wpool", bufs=2))
    pspool = ctx.enter_context(tc.tile_pool(name="pspool", bufs=2, space="PSUM"))
    sgpool = ctx.enter_context(tc.tile_pool(name="sgpool", bufs=3))

    def load_weights(e, first=False):
        w0 = wpool.tile([DIN, 256], BF16, tag="w0", name="w0")
        if first:
            # startup: chunk-0 inputs and the bias table first so the first
            # L0 matmul + silu can start ASAP; then the weight bulk
            nc.sync.dma_start(cns[:], io["cns"])
            nc.sync.dma_start(xt[:, 0:CH], io["xt"][:, 0:CH])
            nc.sync.dma_start(w0[:], io["w0"][e])
            nc.sync.dma_start(xt[:, CH:BL], io["xt"][:, CH:BL])
            nc.sync.dma_start(sgc[:], io["sgc"])
        else:
            nc.sync.dma_start(w0[:], io["w0"][e])
        # DoubleRow weights: [128, 2, M] with k-half k in free block k
        w1 = wpool.tile([128, 2, 256], F8, tag="w1", name="w1")
        nc.sync.dma_start(w1[:], io["w1"][e])
        w2 = wpool.tile([128, 2, 256], F8, tag="w2", name="w2")
        nc.sync.dma_start(w2[:], io["w2"][e])
        w3 = wpool.tile([128, 2, 128], F8, tag="w3", name="w3")
        nc.sync.dma_start(w3[:], io["w3"][e])
        wh = wpool.tile([128, 64], F8, tag="wh", name="wh")
        nc.sync.dma_start(wh[:], io["wh"][e])
        if first:
            # 1 MB residual tensor last: not read until the first head (~15us)
            nc.sync.dma_start(resid[:], io["resid"])
        return w0, w1, w2, w3, wh

    scratch = cpool.tile([1, 8], F32, tag="scratch")
    nc.gpsimd.memset(scratch[:], 0.0)
    nc.scalar.activation(scratch[0:1, 0:8], scratch[0:1, 0:8], act, bias=0.0)

    xt = cpool.tile([DIN, BL], BF16, tag="xt")
    cns = cpool.tile([128, E * NCONST], F32, tag="cns")
    sgc = cpool.tile([128, 2], F32, tag="sgc")
    resid = cpool.tile([64, BL], F32, tag="resid")

    # sigma pre-activations packed 4 ensembles per tile: row 32*(e%4)+i.
    # Both groups share one buffer: group 0 is flushed at e=3 before
    # group 1 (e=4,5) fills it.
    pk = [sgpool.tile([128, BL], F32, tag="pk", name=f"pk{g}", bufs=1)
          for g in range(2)]

    # --- activation buffers ---
    # hA/hB hold 256 features as [128, 2, BL]: feature 128k+p lives at
    # partition p, free offset k*BL. h3 holds 128 features, plain layout.
    hA = hpool.tile([128, 2, BL], F8, tag="hA", name="hA")
    hB = hpool.tile([128, 2, BL], F8, tag="hB", name="hB")
    # h3 double-buffered: heads(e) run deferred during e+1's L1 phase
    # (where the DVE is otherwise idle) while L3(e+1) fills the other half.
    h3b = [hpool.tile([128, BL], F8, tag=f"h3{i}", name=f"h3{i}")
           for i in range(2)]

    def gemm_l0_part(e, w0, c, mt):
        """x[40,BL] bf16 -> hA block mt chunk c = silu(w0[:,mt].T @ x)."""
        ps = pspool.tile([128, CH], F32, tag="ps", name="ps")
        for j in range(CH // NSUB):
            ncol = slice(c * CH + j * NSUB, c * CH + (j + 1) * NSUB)
            nc.tensor.matmul(
                ps[:, j * NSUB:(j + 1) * NSUB],
                w0[:, mt * 128:(mt + 1) * 128],
                xt[:, ncol],
                start=True, stop=True, skip_group_check=True,
            )
        bcol = e * NCONST + mt
        nc.scalar.activation(
            hA[:, mt, c * CH:(c + 1) * CH], ps[:, :],
            act, bias=cns[:, bcol:bcol + 1],
        )

    def gemm_l0(e, w0):
        for c in range(NCHUNK):
            for mt in range(2):
                gemm_l0_part(e, w0, c, mt)

    def gemm_dr_part(h_in, w, m_tiles, h_out, bias_cols, e, c, mt):
        """DoubleRow layer chunk: silu(sum_k w[:,k,mt*128+].T @ h_in).

        Each matmul streams 2 j-slices of 256 via a 4D moving AP (one
        LDWEIGHTS per 512 out-cols; 512-col streams hide the reloads).
        """
        ps = pspool.tile([128, CH], F32, tag="ps", name="ps")
        for j in range(CH // (2 * NDR)):
            ncol = slice(c * CH + j * 2 * NDR,
                         c * CH + (j + 1) * 2 * NDR)
            nc.tensor.matmul(
                ps[:, j * 2 * NDR:(j + 1) * 2 * NDR]
                .rearrange("m (j n) -> m j n", j=2),
                w[:, :, mt * 128:(mt + 1) * 128],
                h_in[:, :, ncol].rearrange("p k (j n) -> p k j n", j=2),
                start=True, stop=True, perf_mode=DR,
                skip_group_check=True,
            )
        bcol = e * NCONST + bias_cols[mt]
        if m_tiles == 2:
            out_ap = h_out[:, mt, c * CH:(c + 1) * CH]
        else:
            out_ap = h_out[:, c * CH:(c + 1) * CH]
        nc.scalar.activation(
            out_ap, ps[:, :], act, bias=cns[:, bcol:bcol + 1],
        )

    def gemm_dr(h_in, w, m_tiles, h_out, bias_cols, e):
        for c in range(NCHUNK):
            for mt in range(m_tiles):
                gemm_dr_part(h_in, w, m_tiles, h_out, bias_cols, e, c, mt)

    def head_chunk(he, h3, wh, base, width, nsplit):
        """Heads for ensemble he."""
        cs = slice(base, base + width)
        ps = pspool.tile([64, width], F32, tag="ps", name="psh")
        for j in range(width // NSUB):
            ncol = slice(base + j * NSUB, base + (j + 1) * NSUB)
            nc.tensor.matmul(
                ps[:, j * NSUB:(j + 1) * NSUB],
                wh[:, :], h3[:, ncol],
                start=True, stop=True,
            )
        # single fused DVE op drains the whole head psum:
        #   rows 0-31:  mu = psum + bmu + state
        #   rows 32-63: y' = psum + (bsig - max) + 0
        bcol = he * NCONST + 7
        hd = sgpool.tile([64, width], F32, tag="hd", name="hd", bufs=3)
        nc.vector.affine_then_add(
            hd[:, :], ps[:, :], resid[:, cs], 1.0,
            cns[0:64, bcol:bcol + 1],
        )
        step = width // nsplit
        for p in range(nsplit):
            pcs = slice(base + p * step, base + (p + 1) * step)
            pls = slice(p * step, (p + 1) * step)
            nc.sync.dma_start(io["mu"][he * 32:(he + 1) * 32, pcs],
                              hd[0:32, pls])
        # sigmoid via tanh (Silu's table set): s = 0.5*tanh(y'/2)+0.5,
        # sigma = tanh*(exp(max)/2) + (exp(min) + exp(max)/2).
        # Members are packed 4-wide so the tanh uses all 128 ACT lanes;
        # groups flush incrementally (he=3: rows 0-128 of group 0; he=5:
        # rows 0-64 of group 1) and the final member takes a direct
        # unpacked path so the kernel tail skips the pack-copy.
        g, r = divmod(he, 4)
        if he == E - 1:
            sg2 = sgpool.tile([64, width], F32, tag="sg2e", name="sg2e",
                              bufs=2)
            nc.scalar.activation(sg2[32:64, :], hd[32:64, :], AF.Tanh,
                                 scale=0.5)
            sg3 = sgpool.tile([64, width], F32, tag="sg3e", name="sg3e",
                              bufs=2)
            nc.vector.tensor_scalar(
                sg3[32:64, :], sg2[32:64, :],
                sgc[32:64, 0:1], sgc[32:64, 1:2],
                mybir.AluOpType.mult, mybir.AluOpType.add,
            )
            for p in range(width // NSUB):
                pcs = slice(base + p * NSUB, base + (p + 1) * NSUB)
                pls = slice(p * NSUB, (p + 1) * NSUB)
                nc.sync.dma_start(io["sig"][he * 32:(he + 1) * 32, pcs],
                                  sg3[32:64, pls])
        else:
            # 32-partition DVE copies may write any quadrant
            nc.vector.tensor_copy(pk[g][r * 32:(r + 1) * 32, cs],
                                  hd[32:64, :])
        if he in (3, 5):
            rows = 128 if he == 3 else 64
            sg2 = sgpool.tile([128, width], F32, tag="sg2", name="sg2",
                              bufs=2)
            nc.scalar.activation(sg2[0:rows, :], pk[g][0:rows, cs],
                                 AF.Tanh, scale=0.5)
            sg3 = sgpool.tile([128, width], F32, tag="sg3", name="sg3",
                              bufs=2)
            nc.vector.tensor_scalar(
                sg3[0:rows, :], sg2[0:rows, :],
                sgc[0:rows, 0:1], sgc[0:rows, 1:2],
                mybir.AluOpType.mult, mybir.AluOpType.add,
            )
            for p in range(width // NSUB):
                pcs = slice(base + p * NSUB, base + (p + 1) * NSUB)
                pls = slice(p * NSUB, (p + 1) * NSUB)
                nc.sync.dma_start(io["sig"][g * 128:g * 128 + rows, pcs],
                                  sg3[0:rows, pls])

    # Ensemble pipeline with DEFERRED heads: ensemble e's heads execute
    # during e+1's L1 phase (DVE idle there), so the e -> e+1 boundary goes
    # straight from L3's silus to L0(e+1)'s without head matmuls between.
    w_cur = None
    prev = None   # (he, h3buf, wh) of the ensemble whose heads are pending
    for e in range(E):
        if e == 0:
            w_cur = load_weights(0, first=True)
            gemm_l0(0, w_cur[0])
            # L1(0)-c0 runs in the prologue; in steady state L1(e)'s c0
            # runs right after L0(e) at the previous ensemble's tail
            gemm_dr_part(hA, w_cur[1], 2, hB, (2, 3), 0, 0, 0)
            gemm_dr_part(hA, w_cur[1], 2, hB, (2, 3), 0, 0, 1)
        w0, w1, w2, w3, wh = w_cur
        h3e = h3b[e % 2]

        # --- L1 chunk 1, deferred heads(e-1) interleaved ---
        gemm_dr_part(hA, w1, 2, hB, (2, 3), e, 1, 0)
        if prev is not None:
            head_chunk(*prev, 0, CH, 1)
        gemm_dr_part(hA, w1, 2, hB, (2, 3), e, 1, 1)
        if prev is not None:
            head_chunk(*prev, CH, CH, 1)
        if e < E - 1:
            w_nxt = load_weights(e + 1)

        gemm_dr(hB, w2, 2, hA, (4, 5), e)           # 256  -> 256
        gemm_dr(hA, w3, 1, h3e, (6,), e)            # 256  -> 128

        if e == E - 1:
            # short tail: the final member's head runs in 1024-wide chunks
            # so the last tanh->scale->DMA chain is half as long
            for q in range(BL // 1024):
                head_chunk(e, h3e, wh, q * 1024, 1024, 1)
        else:
            gemm_l0_part(e + 1, w_nxt[0], 0, 0)
            gemm_l0_part(e + 1, w_nxt[0], 0, 1)
            gemm_dr_part(hA, w_nxt[1], 2, hB, (2, 3), e + 1, 0, 0)
            gemm_l0_part(e + 1, w_nxt[0], 1, 0)
            gemm_dr_part(hA, w_nxt[1], 2, hB, (2, 3), e + 1, 0, 1)
            gemm_l0_part(e + 1, w_nxt[0], 1, 1)
            prev = (e, h3e, wh)
            w_cur = w_nxt


def build_program(act=AF.Silu):
    nc = bacc.Bacc(
        "TRN2", target_bir_lowering=False, debug=False, num_devices=NCORES
    )
    io = {
        "xt": nc.dram_tensor("xt", [DIN, BL], BF16,
                             kind="ExternalInput").ap(),
        "resid": nc.dram_tensor("resid", [64, BL], F32,
                                kind="ExternalInput").ap(),
        "w0": nc.dram_tensor("w0", [E, DIN, 256], BF16,
                             kind="ExternalInput").ap(),
        "w1": nc.dram_tensor("w1", [E, 128, 2, 256], F8,
                             kind="ExternalInput").ap(),
        "w2": nc.dram_tensor("w2", [E, 128, 2, 256], F8,
                             kind="ExternalInput").ap(),
        "w3": nc.dram_tensor("w3", [E, 128, 2, 128], F8,
                             kind="ExternalInput").ap(),
        "wh": nc.dram_tensor("wh", [E, 128, 64], F8,
                             kind="ExternalInput").ap(),
        "cns": nc.dram_tensor("cns", [128, E * NCONST], F32,
                              kind="ExternalInput").ap(),
        "sgc": nc.dram_tensor("sgc", [128, 2], F32, kind="ExternalInput").ap(),
        "mu": nc.dram_tensor("mu", [E * 32, BL], F32,
                             kind="ExternalOutput").ap(),
        "sig": nc.dram_tensor("sig", [E * 32, BL], F32,
                              kind="ExternalOutput").ap(),
    }
    with tile.TileContext(nc) as tc, ExitStack() as ctx:
        _build_kernel(ctx, tc, io, act=act)
    nc.compile()
    return nc


def host_prep(state, action, W0, b0, W1, b1, W2, b2, W3, b3,
              Wmu, bmu, Wsig, bsig, max_logstd, min_logstd):
    """Full inputs -> (shared input map, per-core shard maps)."""
    f = lambda a: np.ascontiguousarray(np.asarray(a), dtype=np.float32)

    def dr(w):
        # [E, 256, M] fp32 -> [E, 128, 2, M] fp8 DoubleRow layout
        w = f(w)
        e, k, m = w.shape
        return np.ascontiguousarray(
            w.reshape(e, 2, 128, m).transpose(0, 2, 1, 3).astype(NP_F8))

    state, action = f(state), f(action)
    xt_full = np.ascontiguousarray(
        np.concatenate([state, action], axis=1).T
    )  # [40, B] fp32
    wh = np.concatenate([f(Wmu), f(Wsig)], axis=2)
    b0, b1, b2, b3 = f(b0), f(b1), f(b2), f(b3)
    bmu, bsig = f(bmu), f(bsig)
    mx, mn = f(max_logstd), f(min_logstd)

    cns = np.zeros((128, E * NCONST), np.float32)
    for e in range(E):
        c = e * NCONST
        cns[:, c + 0] = b0[e, :128]
        cns[:, c + 1] = b0[e, 128:]
        cns[:, c + 2] = b1[e, :128]
        cns[:, c + 3] = b1[e, 128:]
        cns[:, c + 4] = b2[e, :128]
        cns[:, c + 5] = b2[e, 128:]
        cns[:, c + 6] = b3[e, :]
        cns[0:32, c + 7] = bmu[e]
        cns[32:64, c + 7] = bsig[e] - mx   # sigma-head drain bias

    sgc = np.zeros((128, 2), np.float32)
    sgc[:, 0] = np.tile(np.exp(mx) / 2, 4)
    sgc[:, 1] = np.tile(np.exp(mn) + np.exp(mx) / 2, 4)

    shared = {
        "w0": f(W0).astype(NP_BF16), "w1": dr(W1), "w2": dr(W2), "w3": dr(W3),
        "wh": f(wh).astype(NP_F8),
        "cns": cns, "sgc": sgc,
    }
    resid_full = np.zeros((64, B), np.float32)
    resid_full[0:32] = xt_full[0:32]
    xt_store = xt_full.astype(NP_BF16)
    shards = [
        {
            "xt": np.ascontiguousarray(xt_store[:, c * BL:(c + 1) * BL]),
            "resid": np.ascontiguousarray(resid_full[:, c * BL:(c + 1) * BL]),
        }
        for c in range(NCORES)
    ]
    return shared, shards


def host_post(results):
    """Per-core {mu,sig} [E*32, BL] -> (mu [E,B,32], sigma [E,B,32])."""
    mu = np.empty((E, B, 32), np.float32)
    sigma = np.empty((E, B, 32), np.float32)
    for c in range(NCORES):
        bs = slice(c * BL, (c + 1) * BL)
        mu[:, bs, :] = results[c]["mu"].reshape(E, 32, BL).transpose(0, 2, 1)
        sigma[:, bs, :] = results[c]["sig"].reshape(E, 32, BL).transpose(0, 2, 1)
    return mu, sigma


_PROGRAM = None


def _get_program():
    global _PROGRAM
    if _PROGRAM is None:
        _PROGRAM = build_program()
    return _PROGRAM


def kernel(**inputs):
    nc = _get_program()
    shared, shards = host_prep(**inputs)
    in_maps = [{**shared, **shards[c]} for c in range(NCORES)]
    res = run_bass_kernel_spmd(nc, in_maps, list(range(NCORES)))
    return host_post(res.results)


# revision 44
# speedup vs baseline: 1.1040x; 1.1040x over previous
"""Trainium2 Bass kernel for nn_EnsembleDynamicModel.

Ensemble MLP: E=7 members, x=[state(32)|action(8)] -> 256 -> 256 -> 256 -> 128
-> {mu(32), log_sigma(32)} with swish hidden activations, soft-clamped
log_sigma -> sigma=exp(.), and mu += state residual.

Strategy: data-parallel over the batch axis. Each of the 8 NeuronCores gets
B/8 = 4096 batch rows; ensemble weights are replicated. On-chip layout keeps
activations feature-major ([feature, batch]) so the contraction dim of every
GEMM sits on SBUF partitions.

The K=256 layers (L1/L2/L3) run as fp8e4m3 DoubleRow matmuls: weights and
hidden activations are stored fp8, with the two 128-row K-halves side by
side in the free dim ([128, 2, N] access patterns). DoubleRow contracts
K=256 in a single pass at 1 column/cycle - 2x the bf16 rate. L0 (K=40) and
the heads (K=128) gain nothing from DoubleRow and stay bf16/fp8-1x.
Accuracy: fp8 weights+activations on the hidden chain measures ~2e-3
scale-rel error vs the fp32 reference (gate is 2e-2).

Engines:
  PE   : GEMM chain, ~96us/core (vs ~155us bf16). No accumulation chains -
         every matmul is start=stop=True.
  ACT  : swish fused with the bias add (Silu(psum + b)) writing fp8 directly;
         the sigma head's sigmoid runs as Tanh (same ACT table set as Silu).
         ACT is the bottleneck engine (~190us busy): every cycle counts.
  DVE  : one fused affine_then_add drains each head psum (mu = psum + bmu +
         state on rows 0-31, sigma-preact + (bsig-max) on rows 32-63), plus
         the final sigma scale/offset.

The reference's soft_clamp+exp collapses exactly:
    sigma = exp(min) + exp(max) * sigmoid(y - max).

Ensembles are software-pipelined: L0(e+1) is emitted between L3(e) and
head(e) so the PE always has independent matmul work while head psums drain.
"""

import os
import sys
import numpy as np
from contextlib import ExitStack

# concourse ships with the container image (also on PYTHONPATH via axon_site).
for _p in ("/opt/trn_rl_repo", "/root/.axon_site/_ro/trn_rl_repo"):
    if os.path.isdir(_p) and _p not in sys.path:
        sys.path.append(_p)

import ml_dtypes  # noqa: E402
import concourse.bass as bass  # noqa: E402
import concourse.tile as tile  # noqa: E402
import concourse.mybir as mybir  # noqa: E402
from concourse import bacc  # noqa: E402
from concourse.bass_utils import run_bass_kernel_spmd  # noqa: E402
# NOTE: walrus's --enable-ldw-opt=true (LDWEIGHTS dedup) fails to compile
# fp8 weight loads; redundant reloads are hidden instead by streaming 512
# out-cols per DoubleRow matmul (4D moving APs).

F32 = mybir.dt.float32
BF16 = mybir.dt.bfloat16
F8 = mybir.dt.float8e4
AF = mybir.ActivationFunctionType
DR = mybir.MatmulPerfMode.DoubleRow

NP_BF16 = ml_dtypes.bfloat16
NP_F8 = ml_dtypes.float8_e4m3

E = 7
B = 32768
S = 32
A = 8
DIN = S + A            # 40
NCORES = 8
BL = B // NCORES       # 4096 batch rows per core
CH = 2048              # batch chunk per psum tile (4 PSUM banks fp32)
NSUB = 512             # bf16/1x matmul free dim (1 PSUM bank fp32)
NDR = 256              # DoubleRow out free dim (rhs moving = 2*NDR = 512)
NCHUNK = BL // CH      # 2
NCONST = 8             # const columns per ensemble member


def _build_kernel(ctx, tc, io, act=AF.Silu):
    nc = tc.nc
    cpool = ctx.enter_context(tc.tile_pool(name="cpool", bufs=1))
    hpool = ctx.enter_context(tc.tile_pool(name="hpool", bufs=1))
    wpool = ctx.enter_context(tc.tile_pool(name="wpool", bufs=2))
    pspool = ctx.enter_context(tc.tile_pool(name="pspool", bufs=2, space="PSUM"))
    sgpool = ctx.enter_context(tc.tile_pool(name="sgpool", bufs=3))

    def load_weights(e, first=False):
        w0 = wpool.tile([DIN, 256], BF16, tag="w0", name="w0")
        if first:
            # startup: chunk-0 inputs and the bias table first so the first
            # L0 matmul + silu can start ASAP; then the weight bulk
            nc.sync.dma_start(cns[:], io["cns"])
            nc.sync.dma_start(xt[:, 0:CH], io["xt"][:, 0:CH])
            nc.sync.dma_start(w0[:], io["w0"][e])
            nc.sync.dma_start(xt[:, CH:BL], io["xt"][:, CH:BL])
            nc.sync.dma_start(sgc[:], io["sgc"])
        else:
            nc.sync.dma_start(w0[:], io["w0"][e])
        # DoubleRow weights: [128, 2, M] with k-half k in free block k
        w1 = wpool.tile([128, 2, 256], F8, tag="w1", name="w1")
        nc.sync.dma_start(w1[:], io["w1"][e])
        w2 = wpool.tile([128, 2, 256], F8, tag="w2", name="w2")
        nc.sync.dma_start(w2[:], io["w2"][e])
        w3 = wpool.tile([128, 2, 128], F8, tag="w3", name="w3")
        nc.sync.dma_start(w3[:], io["w3"][e])
        wh = wpool.tile([128, 64], F8, tag="wh", name="wh")
        nc.sync.dma_start(wh[:], io["wh"][e])
        if first:
            # 1 MB residual tensor last: not read until the first head (~15us)
            nc.sync.dma_start(resid[:], io["resid"])
        return w0, w1, w2, w3, wh

    scratch = cpool.tile([1, 8], F32, tag="scratch")
    nc.gpsimd.memset(scratch[:], 0.0)
    nc.scalar.activation(scratch[0:1, 0:8], scratch[0:1, 0:8], act, bias=0.0)

    xt = cpool.tile([DIN, BL], BF16, tag="xt")
    cns = cpool.tile([128, E * NCONST], F32, tag="cns")
    sgc = cpool.tile([128, 2], F32, tag="sgc")
    resid = cpool.tile([64, BL], F32, tag="resid")

    # sigma pre-activations packed 4 ensembles per tile: row 32*(e%4)+i.
    # Both groups share one buffer: group 0 is flushed at e=3 before
    # group 1 (e=4,5) fills it.
    pk = [sgpool.tile([128, BL], F32, tag="pk", name=f"pk{g}", bufs=1)
          for g in range(2)]

    # --- activation buffers ---
    # hA/hB hold 256 features as [128, 2, BL]: feature 128k+p lives at
    # partition p, free offset k*BL. h3 holds 128 features, plain layout.
    hA = hpool.tile([128, 2, BL], F8, tag="hA", name="hA")
    hB = hpool.tile([128, 2, BL], F8, tag="hB", name="hB")
    # L2 writes its own tile (not back into hA): with deferred heads the
    # L0(e+1) silus directly follow L3(e), so their hA writes must not
    # carry a WAR hazard against L3's matmul reads.
    hC = hpool.tile([128, 2, BL], F8, tag="hC", name="hC")
    # h3 double-buffered: heads(e) run deferred during e+1's L1 phase
    # (where the DVE is otherwise idle) while L3(e+1) fills the other half.
    h3b = [hpool.tile([128, BL], F8, tag=f"h3{i}", name=f"h3{i}")
           for i in range(2)]

    def gemm_l0_part(e, w0, c, mt):
        """x[40,BL] bf16 -> hA block mt chunk c = silu(w0[:,mt].T @ x)."""
        ps = pspool.tile([128, CH], F32, tag="ps", name="ps")
        for j in range(CH // NSUB):
            ncol = slice(c * CH + j * NSUB, c * CH + (j + 1) * NSUB)
            nc.tensor.matmul(
                ps[:, j * NSUB:(j + 1) * NSUB],
                w0[:, mt * 128:(mt + 1) * 128],
                xt[:, ncol],
                start=True, stop=True, skip_group_check=True,
            )
        bcol = e * NCONST + mt
        nc.scalar.activation(
            hA[:, mt, c * CH:(c + 1) * CH], ps[:, :],
            act, bias=cns[:, bcol:bcol + 1],
        )

    def gemm_l0(e, w0):
        for c in range(NCHUNK):
            for mt in range(2):
                gemm_l0_part(e, w0, c, mt)

    def gemm_dr_part(h_in, w, m_tiles, h_out, bias_cols, e, c, mt):
        """DoubleRow layer chunk: silu(sum_k w[:,k,mt*128+].T @ h_in).

        Each matmul streams 2 j-slices of 256 via a 4D moving AP (one
        LDWEIGHTS per 512 out-cols; 512-col streams hide the reloads).
        """
        ps = pspool.tile([128, CH], F32, tag="ps", name="ps")
        for j in range(CH // (2 * NDR)):
            ncol = slice(c * CH + j * 2 * NDR,
                         c * CH + (j + 1) * 2 * NDR)
            nc.tensor.matmul(
                ps[:, j * 2 * NDR:(j + 1) * 2 * NDR]
                .rearrange("m (j n) -> m j n", j=2),
                w[:, :, mt * 128:(mt + 1) * 128],
                h_in[:, :, ncol].rearrange("p k (j n) -> p k j n", j=2),
                start=True, stop=True, perf_mode=DR,
                skip_group_check=True,
            )
        bcol = e * NCONST + bias_cols[mt]
        if m_tiles == 2:
            out_ap = h_out[:, mt, c * CH:(c + 1) * CH]
        else:
            out_ap = h_out[:, c * CH:(c + 1) * CH]
        nc.scalar.activation(
            out_ap, ps[:, :], act, bias=cns[:, bcol:bcol + 1],
        )

    def gemm_dr(h_in, w, m_tiles, h_out, bias_cols, e):
        for c in range(NCHUNK):
            for mt in range(m_tiles):
                gemm_dr_part(h_in, w, m_tiles, h_out, bias_cols, e, c, mt)

    def head_chunk(he, h3, wh, base, width, nsplit):
        """Heads for ensemble he."""
        cs = slice(base, base + width)
        ps = pspool.tile([64, width], F32, tag="ps", name="psh")
        for j in range(width // NSUB):
            ncol = slice(base + j * NSUB, base + (j + 1) * NSUB)
            nc.tensor.matmul(
                ps[:, j * NSUB:(j + 1) * NSUB],
                wh[:, :], h3[:, ncol],
                start=True, stop=True,
            )
        # single fused DVE op drains the whole head psum:
        #   rows 0-31:  mu = psum + bmu + state
        #   rows 32-63: y' = psum + (bsig - max) + 0
        bcol = he * NCONST + 7
        hd = sgpool.tile([64, width], F32, tag="hd", name="hd", bufs=3)
        nc.vector.affine_then_add(
            hd[:, :], ps[:, :], resid[:, cs], 1.0,
            cns[0:64, bcol:bcol + 1],
        )
        step = width // nsplit
        for p in range(nsplit):
            pcs = slice(base + p * step, base + (p + 1) * step)
            pls = slice(p * step, (p + 1) * step)
            nc.sync.dma_start(io["mu"][he * 32:(he + 1) * 32, pcs],
                              hd[0:32, pls])
        # sigmoid via tanh (Silu's table set): s = 0.5*tanh(y'/2)+0.5,
        # sigma = tanh*(exp(max)/2) + (exp(min) + exp(max)/2).
        # Members are packed 4-wide so the tanh uses all 128 ACT lanes;
        # groups flush incrementally (he=3: rows 0-128 of group 0; he=5:
        # rows 0-64 of group 1) and the final member takes a direct
        # unpacked path so the kernel tail skips the pack-copy.
        g, r = divmod(he, 4)
        if he == E - 1:
            sg2 = sgpool.tile([64, width], F32, tag="sg2e", name="sg2e",
                              bufs=2)
            nc.scalar.activation(sg2[32:64, :], hd[32:64, :], AF.Tanh,
                                 scale=0.5)
            sg3 = sgpool.tile([64, width], F32, tag="sg3e", name="sg3e",
                              bufs=2)
            nc.vector.tensor_scalar(
                sg3[32:64, :], sg2[32:64, :],
                sgc[32:64, 0:1], sgc[32:64, 1:2],
                mybir.AluOpType.mult, mybir.AluOpType.add,
            )
            for p in range(width // NSUB):
                pcs = slice(base + p * NSUB, base + (p + 1) * NSUB)
                pls = slice(p * NSUB, (p + 1) * NSUB)
                nc.sync.dma_start(io["sig"][he * 32:(he + 1) * 32, pcs],
                                  sg3[32:64, pls])
        else:
            # 32-partition DVE copies may write any quadrant
            nc.vector.tensor_copy(pk[g][r * 32:(r + 1) * 32, cs],
                                  hd[32:64, :])
        if he in (3, 5):
            rows = 128 if he == 3 else 64
            sg2 = sgpool.tile([128, width], F32, tag="sg2", name="sg2",
                              bufs=2)
            nc.scalar.activation(sg2[0:rows, :], pk[g][0:rows, cs],
                                 AF.Tanh, scale=0.5)
            sg3 = sgpool.tile([128, width], F32, tag="sg3", name="sg3",
                              bufs=2)
            nc.vector.tensor_scalar(
                sg3[0:rows, :], sg2[0:rows, :],
                sgc[0:rows, 0:1], sgc[0:rows, 1:2],
                mybir.AluOpType.mult, mybir.AluOpType.add,
            )
            for p in range(width // NSUB):
                pcs = slice(base + p * NSUB, base + (p + 1) * NSUB)
                pls = slice(p * NSUB, (p + 1) * NSUB)
                nc.sync.dma_start(io["sig"][g * 128:g * 128 + rows, pcs],
                                  sg3[0:rows, pls])

    # Ensemble pipeline with DEFERRED heads: ensemble e's heads execute
    # during e+1's L1 phase (DVE idle there), so the e -> e+1 boundary goes
    # straight from L3's silus to L0(e+1)'s without head matmuls between.
    w_cur = None
    prev = None   # (he, h3buf, wh) of the ensemble whose heads are pending
    for e in range(E):
        if e == 0:
            w_cur = load_weights(0, first=True)
            gemm_l0(0, w_cur[0])
        w0, w1, w2, w3, wh = w_cur
        h3e = h3b[e % 2]

        # --- L1, deferred heads(e-1) interleaved ---
        gemm_dr_part(hA, w1, 2, hB, (2, 3), e, 0, 0)
        if prev is not None:
            head_chunk(*prev, 0, CH, 1)
        gemm_dr_part(hA, w1, 2, hB, (2, 3), e, 0, 1)
        gemm_dr_part(hA, w1, 2, hB, (2, 3), e, 1, 0)
        if prev is not None:
            head_chunk(*prev, CH, CH, 1)
        gemm_dr_part(hA, w1, 2, hB, (2, 3), e, 1, 1)
        if e < E - 1:
            w_nxt = load_weights(e + 1)

        gemm_dr(hB, w2, 2, hC, (4, 5), e)           # 256  -> 256
        gemm_dr(hC, w3, 1, h3e, (6,), e)            # 256  -> 128

        if e == E - 1:
            # short tail: the final member's head runs in 1024-wide chunks
            # so the last tanh->scale->DMA chain is half as long
            for q in range(BL // 1024):
                head_chunk(e, h3e, wh, q * 1024, 1024, 1)
        else:
            gemm_l0(e + 1, w_nxt[0])
            prev = (e, h3e, wh)
            w_cur = w_nxt


def build_program(act=AF.Silu):
    nc = bacc.Bacc(
        "TRN2", target_bir_lowering=False, debug=False, num_devices=NCORES
    )
    io = {
        "xt": nc.dram_tensor("xt", [DIN, BL], BF16,
                             kind="ExternalInput").ap(),
        "resid": nc.dram_tensor("resid", [64, BL], F32,
                                kind="ExternalInput").ap(),
        "w0": nc.dram_tensor("w0", [E, DIN, 256], BF16,
                             kind="ExternalInput").ap(),
        "w1": nc.dram_tensor("w1", [E, 128, 2, 256], F8,
                             kind="ExternalInput").ap(),
        "w2": nc.dram_tensor("w2", [E, 128, 2, 256], F8,
                             kind="ExternalInput").ap(),
        "w3": nc.dram_tensor("w3", [E, 128, 2, 128], F8,
                             kind="ExternalInput").ap(),
        "wh": nc.dram_tensor("wh", [E, 128, 64], F8,
                             kind="ExternalInput").ap(),
        "cns": nc.dram_tensor("cns", [128, E * NCONST], F32,
                              kind="ExternalInput").ap(),
        "sgc": nc.dram_tensor("sgc", [128, 2], F32, kind="ExternalInput").ap(),
        "mu": nc.dram_tensor("mu", [E * 32, BL], F32,
                             kind="ExternalOutput").ap(),
        "sig": nc.dram_tensor("sig", [E * 32, BL], F32,
                              kind="ExternalOutput").ap(),
    }
    with tile.TileContext(nc) as tc, ExitStack() as ctx:
        _build_kernel(ctx, tc, io, act=act)
    nc.compile()
    return nc


def host_prep(state, action, W0, b0, W1, b1, W2, b2, W3, b3,
              Wmu, bmu, Wsig, bsig, max_logstd, min_logstd):
    """Full inputs -> (shared input map, per-core shard maps)."""
    f = lambda a: np.ascontiguousarray(np.asarray(a), dtype=np.float32)

    def dr(w):
        # [E, 256, M] fp32 -> [E, 128, 2, M] fp8 DoubleRow layout
        w = f(w)
        e, k, m = w.shape
        return np.ascontiguousarray(
            w.reshape(e, 2, 128, m).transpose(0, 2, 1, 3).astype(NP_F8))

    state, action = f(state), f(action)
    xt_full = np.ascontiguousarray(
        np.concatenate([state, action], axis=1).T
    )  # [40, B] fp32
    wh = np.concatenate([f(Wmu), f(Wsig)], axis=2)
    b0, b1, b2, b3 = f(b0), f(b1), f(b2), f(b3)
    bmu, bsig = f(bmu), f(bsig)
    mx, mn = f(max_logstd), f(min_logstd)

    cns = np.zeros((128, E * NCONST), np.float32)
    for e in range(E):
        c = e * NCONST
        cns[:, c + 0] = b0[e, :128]
        cns[:, c + 1] = b0[e, 128:]
        cns[:, c + 2] = b1[e, :128]
        cns[:, c + 3] = b1[e, 128:]
        cns[:, c + 4] = b2[e, :128]
        cns[:, c + 5] = b2[e, 128:]
        cns[:, c + 6] = b3[e, :]
        cns[0:32, c + 7] = bmu[e]
        cns[32:64, c + 7] = bsig[e] - mx   # sigma-head drain bias

    sgc = np.zeros((128, 2), np.float32)
    sgc[:, 0] = np.tile(np.exp(mx) / 2, 4)
    sgc[:, 1] = np.tile(np.exp(mn) + np.exp(mx) / 2, 4)

    shared = {
        "w0": f(W0).astype(NP_BF16), "w1": dr(W1), "w2": dr(W2), "w3": dr(W3),
        "wh": f(wh).astype(NP_F8),
        "cns": cns, "sgc": sgc,
    }
    resid_full = np.zeros((64, B), np.float32)
    resid_full[0:32] = xt_full[0:32]
    xt_store = xt_full.astype(NP_BF16)
    shards = [
        {
            "xt": np.ascontiguousarray(xt_store[:, c * BL:(c + 1) * BL]),
            "resid": np.ascontiguousarray(resid_full[:, c * BL:(c + 1) * BL]),
        }
        for c in range(NCORES)
    ]
    return shared, shards


def host_post(results):
    """Per-core {mu,sig} [E*32, BL] -> (mu [E,B,32], sigma [E,B,32])."""
    mu = np.empty((E, B, 32), np.float32)
    sigma = np.empty((E, B, 32), np.float32)
    for c in range(NCORES):
        bs = slice(c * BL, (c + 1) * BL)
        mu[:, bs, :] = results[c]["mu"].reshape(E, 32, BL).transpose(0, 2, 1)
        sigma[:, bs, :] = results[c]["sig"].reshape(E, 32, BL).transpose(0, 2, 1)
    return mu, sigma


_PROGRAM = None


def _get_program():
    global _PROGRAM
    if _PROGRAM is None:
        _PROGRAM = build_program()
    return _PROGRAM


def kernel(**inputs):
    nc = _get_program()
    shared, shards = host_prep(**inputs)
    in_maps = [{**shared, **shards[c]} for c in range(NCORES)]
    res = run_bass_kernel_spmd(nc, in_maps, list(range(NCORES)))
    return host_post(res.results)


# revision 46
# speedup vs baseline: 1.1080x; 1.0037x over previous
"""Trainium2 Bass kernel for nn_EnsembleDynamicModel.

Ensemble MLP: E=7 members, x=[state(32)|action(8)] -> 256 -> 256 -> 256 -> 128
-> {mu(32), log_sigma(32)} with swish hidden activations, soft-clamped
log_sigma -> sigma=exp(.), and mu += state residual.

Strategy: data-parallel over the batch axis. Each of the 8 NeuronCores gets
B/8 = 4096 batch rows; ensemble weights are replicated. On-chip layout keeps
activations feature-major ([feature, batch]) so the contraction dim of every
GEMM sits on SBUF partitions.

The K=256 layers (L1/L2/L3) run as fp8e4m3 DoubleRow matmuls: weights and
hidden activations are stored fp8, with the two 128-row K-halves side by
side in the free dim ([128, 2, N] access patterns). DoubleRow contracts
K=256 in a single pass at 1 column/cycle - 2x the bf16 rate. L0 (K=40) and
the heads (K=128) gain nothing from DoubleRow and stay bf16/fp8-1x.
Accuracy: fp8 weights+activations on the hidden chain measures ~2e-3
scale-rel error vs the fp32 reference (gate is 2e-2).

Engines:
  PE   : GEMM chain, ~96us/core (vs ~155us bf16). No accumulation chains -
         every matmul is start=stop=True.
  ACT  : swish fused with the bias add (Silu(psum + b)) writing fp8 directly;
         the sigma head's sigmoid runs as Tanh (same ACT table set as Silu).
         ACT is the bottleneck engine (~190us busy): every cycle counts.
  DVE  : one fused affine_then_add drains each head psum (mu = psum + bmu +
         state on rows 0-31, sigma-preact + (bsig-max) on rows 32-63), plus
         the final sigma scale/offset.

The reference's soft_clamp+exp collapses exactly:
    sigma = exp(min) + exp(max) * sigmoid(y - max).

Ensembles are software-pipelined: L0(e+1) is emitted between L3(e) and
head(e) so the PE always has independent matmul work while head psums drain.
"""

import os
import sys
import numpy as np
from contextlib import ExitStack

# concourse ships with the container image (also on PYTHONPATH via axon_site).
for _p in ("/opt/trn_rl_repo", "/root/.axon_site/_ro/trn_rl_repo"):
    if os.path.isdir(_p) and _p not in sys.path:
        sys.path.append(_p)

import ml_dtypes  # noqa: E402
import concourse.bass as bass  # noqa: E402
import concourse.tile as tile  # noqa: E402
import concourse.mybir as mybir  # noqa: E402
from concourse import bacc  # noqa: E402
from concourse.bass_utils import run_bass_kernel_spmd  # noqa: E402
# NOTE: walrus's --enable-ldw-opt=true (LDWEIGHTS dedup) fails to compile
# fp8 weight loads; redundant reloads are hidden instead by streaming 512
# out-cols per DoubleRow matmul (4D moving APs).

F32 = mybir.dt.float32
BF16 = mybir.dt.bfloat16
F8 = mybir.dt.float8e4
AF = mybir.ActivationFunctionType
DR = mybir.MatmulPerfMode.DoubleRow

NP_BF16 = ml_dtypes.bfloat16
NP_F8 = ml_dtypes.float8_e4m3

E = 7
B = 32768
S = 32
A = 8
DIN = S + A            # 40
NCORES = 8
BL = B // NCORES       # 4096 batch rows per core
CH = 2048              # batch chunk per psum tile (4 PSUM banks fp32)
NSUB = 512             # bf16/1x matmul free dim (1 PSUM bank fp32)
NDR = 256              # DoubleRow out free dim (rhs moving = 2*NDR = 512)
NCHUNK = BL // CH      # 2
NCONST = 8             # const columns per ensemble member


def _build_kernel(ctx, tc, io, act=AF.Silu):
    nc = tc.nc
    cpool = ctx.enter_context(tc.tile_pool(name="cpool", bufs=1))
    hpool = ctx.enter_context(tc.tile_pool(name="hpool", bufs=1))
    # bufs=3: deferred heads(e-1) read wh(e-1) during e's L1 phase while
    # load_weights(e+1) runs; a 2-deep rotation would make the wh DMA wait
    # on those head matmuls, stalling the sync queue behind it.
    wpool = ctx.enter_context(tc.tile_pool(name="wpool", bufs=3))
    pspool = ctx.enter_context(tc.tile_pool(name="pspool", bufs=2, space="PSUM"))
    sgpool = ctx.enter_context(tc.tile_pool(name="sgpool", bufs=3))

    def load_weights(e, first=False):
        w0 = wpool.tile([DIN, 256], BF16, tag="w0", name="w0")
        if first:
            # startup: chunk-0 inputs and the bias table first so the first
            # L0 matmul + silu can start ASAP; then the weight bulk
            nc.sync.dma_start(cns[:], io["cns"])
            nc.sync.dma_start(xt[:, 0:CH], io["xt"][:, 0:CH])
            nc.sync.dma_start(w0[:], io["w0"][e])
            nc.sync.dma_start(xt[:, CH:BL], io["xt"][:, CH:BL])
            nc.sync.dma_start(sgc[:], io["sgc"])
        else:
            nc.sync.dma_start(w0[:], io["w0"][e])
        # DoubleRow weights: [128, 2, M] with k-half k in free block k
        w1 = wpool.tile([128, 2, 256], F8, tag="w1", name="w1")
        nc.sync.dma_start(w1[:], io["w1"][e])
        w2 = wpool.tile([128, 2, 256], F8, tag="w2", name="w2")
        nc.sync.dma_start(w2[:], io["w2"][e])
        w3 = wpool.tile([128, 2, 128], F8, tag="w3", name="w3")
        nc.sync.dma_start(w3[:], io["w3"][e])
        wh = wpool.tile([128, 64], F8, tag="wh", name="wh")
        nc.sync.dma_start(wh[:], io["wh"][e])
        if first:
            # 1 MB residual tensor last: not read until the first head (~15us)
            nc.sync.dma_start(resid[:], io["resid"])
        return w0, w1, w2, w3, wh

    scratch = cpool.tile([1, 8], F32, tag="scratch")
    nc.gpsimd.memset(scratch[:], 0.0)
    nc.scalar.activation(scratch[0:1, 0:8], scratch[0:1, 0:8], act, bias=0.0)

    xt = cpool.tile([DIN, BL], BF16, tag="xt")
    cns = cpool.tile([128, E * NCONST], F32, tag="cns")
    sgc = cpool.tile([128, 2], F32, tag="sgc")
    resid = cpool.tile([64, BL], F32, tag="resid")

    # sigma pre-activations packed 4 ensembles per tile: row 32*(e%4)+i.
    # Both groups share one buffer: group 0 is flushed at e=3 before
    # group 1 (e=4,5) fills it.
    pk = [sgpool.tile([128, BL], F32, tag="pk", name=f"pk{g}", bufs=1)
          for g in range(2)]

    # --- activation buffers ---
    # hA/hB hold 256 features as [128, 2, BL]: feature 128k+p lives at
    # partition p, free offset k*BL. h3 holds 128 features, plain layout.
    hA = hpool.tile([128, 2, BL], F8, tag="hA", name="hA")
    hB = hpool.tile([128, 2, BL], F8, tag="hB", name="hB")
    # h3 double-buffered: heads(e) run deferred during e+1's L1 phase
    # (where the DVE is otherwise idle) while L3(e+1) fills the other half.
    h3b = [hpool.tile([128, BL], F8, tag=f"h3{i}", name=f"h3{i}")
           for i in range(2)]

    def gemm_l0_part(e, w0, c, mt):
        """x[40,BL] bf16 -> hA block mt chunk c = silu(w0[:,mt].T @ x)."""
        ps = pspool.tile([128, CH], F32, tag="ps", name="ps")
        for j in range(CH // NSUB):
            ncol = slice(c * CH + j * NSUB, c * CH + (j + 1) * NSUB)
            nc.tensor.matmul(
                ps[:, j * NSUB:(j + 1) * NSUB],
                w0[:, mt * 128:(mt + 1) * 128],
                xt[:, ncol],
                start=True, stop=True, skip_group_check=True,
            )
        bcol = e * NCONST + mt
        nc.scalar.activation(
            hA[:, mt, c * CH:(c + 1) * CH], ps[:, :],
            act, bias=cns[:, bcol:bcol + 1],
        )

    def gemm_l0(e, w0):
        for c in range(NCHUNK):
            for mt in range(2):
                gemm_l0_part(e, w0, c, mt)

    def gemm_dr_part(h_in, w, m_tiles, h_out, bias_cols, e, c, mt):
        """DoubleRow layer chunk: silu(sum_k w[:,k,mt*128+].T @ h_in).

        Each matmul streams 2 j-slices of 256 via a 4D moving AP (one
        LDWEIGHTS per 512 out-cols; 512-col streams hide the reloads).
        """
        ps = pspool.tile([128, CH], F32, tag="ps", name="ps")
        for j in range(CH // (2 * NDR)):
            ncol = slice(c * CH + j * 2 * NDR,
                         c * CH + (j + 1) * 2 * NDR)
            nc.tensor.matmul(
                ps[:, j * 2 * NDR:(j + 1) * 2 * NDR]
                .rearrange("m (j n) -> m j n", j=2),
                w[:, :, mt * 128:(mt + 1) * 128],
                h_in[:, :, ncol].rearrange("p k (j n) -> p k j n", j=2),
                start=True, stop=True, perf_mode=DR,
                skip_group_check=True,
            )
        bcol = e * NCONST + bias_cols[mt]
        if m_tiles == 2:
            out_ap = h_out[:, mt, c * CH:(c + 1) * CH]
        else:
            out_ap = h_out[:, c * CH:(c + 1) * CH]
        nc.scalar.activation(
            out_ap, ps[:, :], act, bias=cns[:, bcol:bcol + 1],
        )

    def gemm_dr(h_in, w, m_tiles, h_out, bias_cols, e):
        for c in range(NCHUNK):
            for mt in range(m_tiles):
                gemm_dr_part(h_in, w, m_tiles, h_out, bias_cols, e, c, mt)

    def head_chunk(he, h3, wh, base, width, nsplit):
        """Heads for ensemble he."""
        cs = slice(base, base + width)
        ps = pspool.tile([64, width], F32, tag="ps", name="psh")
        for j in range(width // NSUB):
            ncol = slice(base + j * NSUB, base + (j + 1) * NSUB)
            nc.tensor.matmul(
                ps[:, j * NSUB:(j + 1) * NSUB],
                wh[:, :], h3[:, ncol],
                start=True, stop=True,
            )
        # single fused DVE op drains the whole head psum:
        #   rows 0-31:  mu = psum + bmu + state
        #   rows 32-63: y' = psum + (bsig - max) + 0
        bcol = he * NCONST + 7
        hd = sgpool.tile([64, width], F32, tag="hd", name="hd", bufs=3)
        nc.vector.affine_then_add(
            hd[:, :], ps[:, :], resid[:, cs], 1.0,
            cns[0:64, bcol:bcol + 1],
        )
        step = width // nsplit
        for p in range(nsplit):
            pcs = slice(base + p * step, base + (p + 1) * step)
            pls = slice(p * step, (p + 1) * step)
            nc.sync.dma_start(io["mu"][he * 32:(he + 1) * 32, pcs],
                              hd[0:32, pls])
        # sigmoid via tanh (Silu's table set): s = 0.5*tanh(y'/2)+0.5,
        # sigma = tanh*(exp(max)/2) + (exp(min) + exp(max)/2).
        # Members are packed 4-wide so the tanh uses all 128 ACT lanes;
        # groups flush incrementally (he=3: rows 0-128 of group 0; he=5:
        # rows 0-64 of group 1) and the final member takes a direct
        # unpacked path so the kernel tail skips the pack-copy.
        g, r = divmod(he, 4)
        if he == E - 1:
            sg2 = sgpool.tile([64, width], F32, tag="sg2e", name="sg2e",
                              bufs=3)
            nc.scalar.activation(sg2[32:64, :], hd[32:64, :], AF.Tanh,
                                 scale=0.5)
            sg3 = sgpool.tile([64, width], F32, tag="sg3e", name="sg3e",
                              bufs=3)
            nc.vector.tensor_scalar(
                sg3[32:64, :], sg2[32:64, :],
                sgc[32:64, 0:1], sgc[32:64, 1:2],
                mybir.AluOpType.mult, mybir.AluOpType.add,
            )
            for p in range(width // NSUB):
                pcs = slice(base + p * NSUB, base + (p + 1) * NSUB)
                pls = slice(p * NSUB, (p + 1) * NSUB)
                nc.sync.dma_start(io["sig"][he * 32:(he + 1) * 32, pcs],
                                  sg3[32:64, pls])
        else:
            # 32-partition DVE copies may write any quadrant
            nc.vector.tensor_copy(pk[g][r * 32:(r + 1) * 32, cs],
                                  hd[32:64, :])
        if he in (3, 5):
            rows = 128 if he == 3 else 64
            sg2 = sgpool.tile([128, width], F32, tag="sg2", name="sg2",
                              bufs=2)
            nc.scalar.activation(sg2[0:rows, :], pk[g][0:rows, cs],
                                 AF.Tanh, scale=0.5)
            sg3 = sgpool.tile([128, width], F32, tag="sg3", name="sg3",
                              bufs=2)
            nc.vector.tensor_scalar(
                sg3[0:rows, :], sg2[0:rows, :],
                sgc[0:rows, 0:1], sgc[0:rows, 1:2],
                mybir.AluOpType.mult, mybir.AluOpType.add,
            )
            for p in range(width // NSUB):
                pcs = slice(base + p * NSUB, base + (p + 1) * NSUB)
                pls = slice(p * NSUB, (p + 1) * NSUB)
                nc.sync.dma_start(io["sig"][g * 128:g * 128 + rows, pcs],
                                  sg3[0:rows, pls])

    # Ensemble pipeline with DEFERRED heads: ensemble e's heads execute
    # during e+1's L1 phase (DVE idle there), so the e -> e+1 boundary goes
    # straight from L3's silus to L0(e+1)'s without head matmuls between.
    w_cur = None
    prev = None   # (he, h3buf, wh) of the ensemble whose heads are pending
    for e in range(E):
        if e == 0:
            w_cur = load_weights(0, first=True)
            gemm_l0(0, w_cur[0])
        w0, w1, w2, w3, wh = w_cur
        h3e = h3b[e % 2]

        # --- L1, deferred heads(e-1) interleaved ---
        gemm_dr_part(hA, w1, 2, hB, (2, 3), e, 0, 0)
        if prev is not None:
            head_chunk(*prev, 0, CH, 1)
        gemm_dr_part(hA, w1, 2, hB, (2, 3), e, 0, 1)
        gemm_dr_part(hA, w1, 2, hB, (2, 3), e, 1, 0)
        if prev is not None:
            head_chunk(*prev, CH, CH, 1)
        gemm_dr_part(hA, w1, 2, hB, (2, 3), e, 1, 1)
        if e < E - 1:
            w_nxt = load_weights(e + 1)

        gemm_dr(hB, w2, 2, hA, (4, 5), e)           # 256  -> 256
        gemm_dr(hA, w3, 1, h3e, (6,), e)            # 256  -> 128

        if e == E - 1:
            # short tail: the final member's head runs in 1024-wide chunks
            # so the last tanh->scale->DMA chain is half as long
            for q in range(BL // 1024):
                head_chunk(e, h3e, wh, q * 1024, 1024, 1)
        else:
            gemm_l0(e + 1, w_nxt[0])
            prev = (e, h3e, wh)
            w_cur = w_nxt


def build_program(act=AF.Silu):
    nc = bacc.Bacc(
        "TRN2", target_bir_lowering=False, debug=False, num_devices=NCORES
    )
    io = {
        "xt": nc.dram_tensor("xt", [DIN, BL], BF16,
                             kind="ExternalInput").ap(),
        "resid": nc.dram_tensor("resid", [64, BL], F32,
                                kind="ExternalInput").ap(),
        "w0": nc.dram_tensor("w0", [E, DIN, 256], BF16,
                             kind="ExternalInput").ap(),
        "w1": nc.dram_tensor("w1", [E, 128, 2, 256], F8,
                             kind="ExternalInput").ap(),
        "w2": nc.dram_tensor("w2", [E, 128, 2, 256], F8,
                             kind="ExternalInput").ap(),
        "w3": nc.dram_tensor("w3", [E, 128, 2, 128], F8,
                             kind="ExternalInput").ap(),
        "wh": nc.dram_tensor("wh", [E, 128, 64], F8,
                             kind="ExternalInput").ap(),
        "cns": nc.dram_tensor("cns", [128, E * NCONST], F32,
                              kind="ExternalInput").ap(),
        "sgc": nc.dram_tensor("sgc", [128, 2], F32, kind="ExternalInput").ap(),
        "mu": nc.dram_tensor("mu", [E * 32, BL], F32,
                             kind="ExternalOutput").ap(),
        "sig": nc.dram_tensor("sig", [E * 32, BL], F32,
                              kind="ExternalOutput").ap(),
    }
    with tile.TileContext(nc) as tc, ExitStack() as ctx:
        _build_kernel(ctx, tc, io, act=act)
    nc.compile()
    return nc


def host_prep(state, action, W0, b0, W1, b1, W2, b2, W3, b3,
              Wmu, bmu, Wsig, bsig, max_logstd, min_logstd):
    """Full inputs -> (shared input map, per-core shard maps)."""
    f = lambda a: np.ascontiguousarray(np.asarray(a), dtype=np.float32)

    def dr(w):
        # [E, 256, M] fp32 -> [E, 128, 2, M] fp8 DoubleRow layout
        w = f(w)
        e, k, m = w.shape
        return np.ascontiguousarray(
            w.reshape(e, 2, 128, m).transpose(0, 2, 1, 3).astype(NP_F8))

    state, action = f(state), f(action)
    xt_full = np.ascontiguousarray(
        np.concatenate([state, action], axis=1).T
    )  # [40, B] fp32
    wh = np.concatenate([f(Wmu), f(Wsig)], axis=2)
    b0, b1, b2, b3 = f(b0), f(b1), f(b2), f(b3)
    bmu, bsig = f(bmu), f(bsig)
    mx, mn = f(max_logstd), f(min_logstd)

    cns = np.zeros((128, E * NCONST), np.float32)
    for e in range(E):
        c = e * NCONST
        cns[:, c + 0] = b0[e, :128]
        cns[:, c + 1] = b0[e, 128:]
        cns[:, c + 2] = b1[e, :128]
        cns[:, c + 3] = b1[e, 128:]
        cns[:, c + 4] = b2[e, :128]
        cns[:, c + 5] = b2[e, 128:]
        cns[:, c + 6] = b3[e, :]
        cns[0:32, c + 7] = bmu[e]
        cns[32:64, c + 7] = bsig[e] - mx   # sigma-head drain bias

    sgc = np.zeros((128, 2), np.float32)
    sgc[:, 0] = np.tile(np.exp(mx) / 2, 4)
    sgc[:, 1] = np.tile(np.exp(mn) + np.exp(mx) / 2, 4)

    shared = {
        "w0": f(W0).astype(NP_BF16), "w1": dr(W1), "w2": dr(W2), "w3": dr(W3),
        "wh": f(wh).astype(NP_F8),
        "cns": cns, "sgc": sgc,
    }
    resid_full = np.zeros((64, B), np.float32)
    resid_full[0:32] = xt_full[0:32]
    xt_store = xt_full.astype(NP_BF16)
    shards = [
        {
            "xt": np.ascontiguousarray(xt_store[:, c * BL:(c + 1) * BL]),
            "resid": np.ascontiguousarray(resid_full[:, c * BL:(c + 1) * BL]),
        }
        for c in range(NCORES)
    ]
    return shared, shards


def host_post(results):
    """Per-core {mu,sig} [E*32, BL] -> (mu [E,B,32], sigma [E,B,32])."""
    mu = np.empty((E, B, 32), np.float32)
    sigma = np.empty((E, B, 32), np.float32)
    for c in range(NCORES):
        bs = slice(c * BL, (c + 1) * BL)
        mu[:, bs, :] = results[c]["mu"].reshape(E, 32, BL).transpose(0, 2, 1)
        sigma[:, bs, :] = results[c]["sig"].reshape(E, 32, BL).transpose(0, 2, 1)
    return mu, sigma


_PROGRAM = None


def _get_program():
    global _PROGRAM
    if _PROGRAM is None:
        _PROGRAM = build_program()
    return _PROGRAM


def kernel(**inputs):
    nc = _get_program()
    shared, shards = host_prep(**inputs)
    in_maps = [{**shared, **shards[c]} for c in range(NCORES)]
    res = run_bass_kernel_spmd(nc, in_maps, list(range(NCORES)))
    return host_post(res.results)
